# revision 6
# baseline (speedup 1.0000x reference)
"""Cross-attention Trainium2 Bass kernel (bf16 rewrite).

Sharding: data-parallel over batch — 16 batches across 8 cores, 2 per core.
Weights replicated. Each core computes its 2 batches fully; no collectives.

All matmuls run in bf16 (1 cycle/row at any moving size on TRN2's PE).
PE transposes are eliminated entirely: x and attn are transposed by the
DMA crossbar (dma_start_transpose, 2-byte dtype, 16x128 xbar tiles), whose
destination mapping is out[p, c, f] = in[f, c*128 + p] (verified on hw).

Per-core dataflow, per batch b / 512-row x tile:
  x_bf   = bf16(x tile)                 (SWDGE casting DMA, Pool engine)
  xT     = DMA-transpose(x_bf)          -> [d, s] layout
  qT     = Wq^T @ xT                    (PE, 4x4 mm; PSUM->SBUF copy)
  per head h:  scT = kT_h^T @ qT_h      [77, 512] (PE)
               et_h = exp(0.125 * scT)  (ACT, bf16 out)
  per s-chunk c (128 rows), per 4-head group:
    pa[:, hh*66:+66] = et_h_chunk^T @ [v_h | 1 | *]   (PE; col 64 = denom)
    rr = 1/pa[:, :, 64]                 (DVE)
    attn_n = pa[:, :, 0:64] * rr        (DVE, bf16, per-partition scalar)
  attnT  = DMA-transpose(attn_n)        -> [e, s] layout
  out    = attnT^T @ Wout (+ bout via rank-1 ones matmul, accumulated
           in PSUM), stored to DRAM directly from PSUM by SWDGE DMA.

Weights are cast-loaded fp32->bf16 by SWDGE DMA, laid out
"(c p) e -> p c e" so partition p of chunk c holds row c*128+p, matching
the DMA-transpose output mapping. TRN2 allows 1 semaphore wait per
instruction — generate_event_semaphores() legalizes multi-wait
instructions that Tile emits.
"""

import numpy as np

import bass_rust as _bass_rust
import concourse.bass as bass
import concourse.mybir as mybir
import concourse.tile as tile
from concourse.bass import broadcast_tensor_aps
from concourse.bass_utils import run_bass_kernel_spmd

N_CORES = 8
B, SQ, DM = 16, 4096, 512
SKV, DC = 77, 768
H, DH = 8, 64
INNER = 512
BPC = B // N_CORES  # batches per core
NT = SQ // 512      # x tiles per batch

F32 = mybir.dt.float32
BF16 = mybir.dt.bfloat16

AF = mybir.ActivationFunctionType


def build_nc(trace_sim=False):
    nc = bass.Bass()

    x_d = nc.dram_tensor("x", [BPC, SQ, DM], F32, kind="ExternalInput")
    ctx_d = nc.dram_tensor("context", [BPC, SKV, DC], F32, kind="ExternalInput")
    wq_d = nc.dram_tensor("Wq", [DM, INNER], F32, kind="ExternalInput")
    wk_d = nc.dram_tensor("Wk", [DC, INNER], F32, kind="ExternalInput")
    wv_d = nc.dram_tensor("Wv", [DC, INNER], F32, kind="ExternalInput")
    wo_d = nc.dram_tensor("Wout", [INNER, INNER], F32, kind="ExternalInput")
    bo_d = nc.dram_tensor("bout", [INNER], F32, kind="ExternalInput")
    out_d = nc.dram_tensor("out", [BPC, SQ, DM], F32, kind="ExternalOutput")

    with tile.TileContext(nc, trace_sim=trace_sim) as tc:
        with (
            tc.tile_pool(name="const", bufs=1) as consts,
            tc.tile_pool(name="kvp", bufs=2) as kvp,
            tc.tile_pool(name="xload", bufs=3) as xload,
            tc.tile_pool(name="xtp", bufs=2) as xtp,
            tc.tile_pool(name="qtp", bufs=2) as qtp,
            tc.tile_pool(name="etp", bufs=2) as etp,
            tc.tile_pool(name="rrp", bufs=8) as rrp,
            tc.tile_pool(name="anp", bufs=2) as anp,
            tc.tile_pool(name="atp", bufs=2) as atp,
            tc.tile_pool(name="osp", bufs=2) as osp,
            tc.tile_pool(name="pbig", bufs=3, space="PSUM") as pbig,
            tc.tile_pool(name="psc", bufs=2, space="PSUM") as psc,
            tc.tile_pool(name="pau", bufs=3, space="PSUM") as pau,
        ):
            # ---- weights: casting SWDGE loads, fp32 DRAM -> bf16 SBUF ----
            # layout "(c p) e -> p c e": partition p of chunk c holds row
            # c*128+p — same mapping as the DMA-transpose destination.
            wk_sb = consts.tile([128, DC // 128, INNER], BF16, tag="wk")
            nc.gpsimd.dma_start(out=wk_sb, in_=wk_d[:].rearrange("(c p) e -> p c e", p=128))
            wv_sb = consts.tile([128, DC // 128, INNER], BF16, tag="wv")
            nc.gpsimd.dma_start(out=wv_sb, in_=wv_d[:].rearrange("(c p) e -> p c e", p=128))
            wq_sb = consts.tile([128, DM // 128, INNER], BF16, tag="wq")
            nc.gpsimd.dma_start(out=wq_sb, in_=wq_d[:].rearrange("(c p) e -> p c e", p=128))
            wo_sb = consts.tile([128, INNER // 128, INNER], BF16, tag="wo")
            nc.gpsimd.dma_start(out=wo_sb, in_=wo_d[:].rearrange("(c p) e -> p c e", p=128))

            # rank-1 bias trick operands: ones [1, 128] lhsT, bout row [1, 512]
            ones_row = consts.tile([1, 128], BF16, tag="ones_row")
            nc.vector.memset(ones_row, 1.0)
            bo_row = consts.tile([1, INNER], BF16, tag="bo_row")
            nc.gpsimd.dma_start(out=bo_row, in_=bo_d[:].rearrange("(p e) -> p e", p=1))

            def emit_kv(b):
                # ctx cast-load into a 80-partition tile (pad rows 77..79 are
                # never read downstream; DMA-T needs p % 16 == 0)
                ctx_bf = kvp.tile([80, DC], BF16, tag="ctx")
                nc.gpsimd.dma_start(out=ctx_bf[0:SKV, :], in_=ctx_d[b])
                ctxT = kvp.tile([128, DC // 128, 80], BF16, tag="ctxT")
                nc.sync.dma_start_transpose(out=ctxT, in_=ctx_bf[:, :])

                # kT[e, kv]: lhsT = Wk chunk, rhs = ctxT chunk
                kT_sb = kvp.tile([128, INNER // 128, SKV], BF16, tag="kT")
                for i in range(INNER // 128):
                    pk = pbig.tile([128, 512], F32, tag="big")
                    for j in range(DC // 128):
                        nc.tensor.matmul(
                            out=pk[:, 0:SKV],
                            lhsT=wk_sb[:, j, i * 128:(i + 1) * 128],
                            rhs=ctxT[:, j, 0:SKV],
                            start=(j == 0), stop=(j == DC // 128 - 1),
                        )
                    nc.scalar.copy(out=kT_sb[:, i, :], in_=pk[:, 0:SKV])

                # v[kv, e] with an appended ones column per head:
                # v_aug[:, h, 0:64] = v_h, v_aug[:, h, 64] = 1 (denominator)
                pv = pbig.tile([128, 512], F32, tag="big")
                for j in range(DC // 128):
                    nc.tensor.matmul(
                        out=pv[0:SKV, :],
                        lhsT=ctxT[:, j, 0:SKV],
                        rhs=wv_sb[:, j, :],
                        start=(j == 0), stop=(j == DC // 128 - 1),
                    )
                v_aug = kvp.tile([SKV, H, 66], BF16, tag="v_aug")
                nc.scalar.copy(
                    out=v_aug[:, :, 0:64],
                    in_=pv[0:SKV, :].rearrange("p (h d) -> p h d", h=H),
                )
                nc.vector.memset(v_aug[:, :, 64:66], 1.0)
                return kT_sb, v_aug

            def emit_outproj(attnT, b, s0):
                # out[s, e'] = attn @ Wout + bout; bias folded in as a
                # rank-1 accumulate so the PSUM->SBUF move is a plain copy
                osb = osp.tile([128, 4, 512], F32, tag="osb")
                for c in range(4):
                    po = pbig.tile([128, 512], F32, tag="big")
                    for j in range(4):
                        nc.tensor.matmul(
                            out=po,
                            lhsT=attnT[:, c, j, :],
                            rhs=wo_sb[:, j, :],
                            start=(j == 0), stop=False,
                        )
                    nc.tensor.matmul(
                        out=po, lhsT=ones_row, rhs=bo_row,
                        start=False, stop=True,
                    )
                    if c % 2 == 0:
                        nc.scalar.copy(out=osb[:, c, :], in_=po)
                    else:
                        nc.vector.tensor_copy(osb[:, c, :], po)
                nc.gpsimd.dma_start(
                    out=out_d[b, s0:s0 + 512, :].rearrange("(t p) d -> p t d", p=128),
                    in_=osb,
                )

            prev = None
            kv = emit_kv(0)
            for b in range(BPC):
                kT_sb, v_aug = kv
                for st in range(NT):
                    if st == 5 and b + 1 < BPC:
                        kv = emit_kv(b + 1)
                    s0 = st * 512

                    # ---- x tile: casting load + DMA transposes ----
                    x_bf = xload.tile([128, 4, DM], BF16, tag="x")
                    nc.gpsimd.dma_start(
                        out=x_bf,
                        in_=x_d[b, s0:s0 + 512, :].rearrange("(t p) d -> p t d", p=128),
                    )
                    xT = xtp.tile([128, 4, 4, 128], BF16, tag="xT")
                    for t in range(4):
                        nc.sync.dma_start_transpose(
                            out=xT[:, t, :, :], in_=x_bf[:, t, :],
                        )

                    # ---- out projection of the PREVIOUS tile (lag-1) ----
                    if prev is not None:
                        emit_outproj(*prev)
                        prev = None

                    # ---- qT = Wq^T @ xT ----
                    qT = qtp.tile([128, 4, 512], BF16, tag="qT")
                    for i in range(4):
                        pq = pbig.tile([128, 512], F32, tag="big")
                        for j in range(4):
                            nc.tensor.matmul(
                                out=pq,
                                lhsT=wq_sb[:, j, i * 128:(i + 1) * 128],
                                rhs=xT[:, :, j, :],
                                start=(j == 0), stop=(j == 3),
                            )
                        if i % 2 == 0:
                            nc.scalar.copy(out=qT[:, i, :], in_=pq)
                        else:
                            nc.vector.tensor_copy(qT[:, i, :], pq)

                    # ---- scores + exp per head ----
                    et = etp.tile([SKV, H, 512], BF16, tag="et")
                    for h in range(H):
                        i, r0 = h // 2, (h % 2) * 64
                        ps = psc.tile([SKV, 512], F32, tag="sc")
                        nc.tensor.matmul(
                            out=ps,
                            lhsT=kT_sb[r0:r0 + 64, i, :],
                            rhs=qT[r0:r0 + 64, i, :],
                            start=True, stop=True,
                        )
                        nc.scalar.activation(
                            out=et[:, h, :], in_=ps, func=AF.Exp, scale=0.125,
                        )

                    # ---- attention values + normalize, per s-chunk ----
                    attn_n = anp.tile([128, 4, 512], BF16, tag="attn_n")
                    attnT = atp.tile([128, 4, 4, 128], BF16, tag="attnT")
                    for c in range(4):
                        for g in range(2):
                            pa = pau.tile([128, 4, 66], F32, tag="attnU")
                            for hh in range(4):
                                h = g * 4 + hh
                                nc.tensor.matmul(
                                    out=pa[:, hh, :],
                                    lhsT=et[:, h, c * 128:(c + 1) * 128],
                                    rhs=v_aug[:, h, :],
                                    start=True, stop=True,
                                )
                            rr = rrp.tile([128, 4, 1], F32, tag="rr")
                            nc.vector.reciprocal(out=rr, in_=pa[:, :, 64:65])
                            out_ap = attn_n[:, c, g * 256:(g + 1) * 256].rearrange(
                                "p (h d) -> p h d", h=4
                            )
                            in0, in1 = broadcast_tensor_aps(pa[:, :, 0:64], rr)
                            nc.vector.tensor_mul(out_ap, in0, in1)
                        nc.sync.dma_start_transpose(
                            out=attnT[:, c, :, :], in_=attn_n[:, c, :],
                        )

                    prev = (attnT, b, s0)

            if prev is not None:
                emit_outproj(*prev)

    # TRN2 hardware allows at most 1 semaphore wait per instruction; split
    # multi-wait instructions into standalone EventSemaphore waits.
    _bass_rust.generate_event_semaphores(nc)
    return nc


_NC_CACHE = None


def kernel(x, context, Wq, Wk, Wv, Wout, bout):
    global _NC_CACHE
    if _NC_CACHE is None:
        _NC_CACHE = build_nc()
    nc = _NC_CACHE

    f = lambda a: np.ascontiguousarray(np.asarray(a), dtype=np.float32)
    x, context = f(x), f(context)
    Wq, Wk, Wv, Wout, bout = f(Wq), f(Wk), f(Wv), f(Wout), f(bout)

    in_maps = [
        {
            "x": x[c * BPC:(c + 1) * BPC],
            "context": context[c * BPC:(c + 1) * BPC],
            "Wq": Wq, "Wk": Wk, "Wv": Wv, "Wout": Wout, "bout": bout,
        }
        for c in range(N_CORES)
    ]
    res = run_bass_kernel_spmd(nc, in_maps, core_ids=list(range(N_CORES)))
    return np.concatenate([r["out"] for r in res.results], axis=0)


# revision 8
# speedup vs baseline: 1.4735x; 1.4735x over previous
"""Cross-attention Trainium2 Bass kernel (bf16, software-pipelined).

Sharding: data-parallel over batch — 16 batches across 8 cores, 2 per core.
Weights replicated. Each core computes its 2 batches fully; no collectives.

All matmuls run in bf16 (1 cycle/row at any moving size on TRN2's PE).
PE transposes are eliminated entirely: x and attn are transposed by the
DMA crossbar (dma_start_transpose, 2-byte dtype, 16x128 xbar tiles), whose
destination mapping is out[p, c, f] = in[f, c*128 + p] (verified on hw).

Per 512-row x tile:
  x_bf   = bf16(x tile)                 (SWDGE casting DMA, Pool engine)
  xT     = DMA-transpose(x_bf)          -> [d, s] layout
  qT     = Wq^T @ xT                    (PE; PSUM->SBUF copy on ACT)
  per head h:  scT = kT_h^T @ qT_h      [77, 512] (PE)
               et_h = exp(0.125 * scT)  (ACT, bf16 out)
  per s-chunk c (128 rows), per 4-head group:
    pa[:, hh, :] = et_h_chunk^T @ [v_h | 1 | 1]   (PE; col 64 = softmax denom)
    rr = 1/pa[:, :, 64]                 (DVE)
    attn_n = pa[:, :, 0:64] * rr        (DVE, bf16, per-partition scalar)
  attnT  = DMA-transpose(attn_n)        -> [e, s] layout
  out    = attnT^T @ Wout + bout        (PE; bias as rank-1 ones matmul
           accumulated in PSUM; plain PSUM->SBUF copy on DVE; SWDGE store)

The per-engine instruction streams are software-pipelined with a 2-tile
lag so no engine waits on same-tile producers:
  iteration k (PE order): qT(k) | scores(k) | attnU(k-1) | outproj(k-2)
with x loads / DMA transposes issued 1-2 iterations ahead.

Weights are cast-loaded fp32->bf16 by SWDGE DMA, laid out
"(c p) e -> p c e" so partition p of chunk c holds row c*128+p, matching
the DMA-transpose output mapping. TRN2 allows 1 semaphore wait per
instruction — generate_event_semaphores() legalizes multi-wait
instructions that Tile emits.
"""

import numpy as np

import bass_rust as _bass_rust
import concourse.bass as bass
import concourse.mybir as mybir
import concourse.tile as tile
from concourse.bass import broadcast_tensor_aps
from concourse.bass_utils import run_bass_kernel_spmd

N_CORES = 8
B, SQ, DM = 16, 4096, 512
SKV, DC = 77, 768
H, DH = 8, 64
INNER = 512
BPC = B // N_CORES  # batches per core
NT = SQ // 512      # x tiles per batch
NTILES = BPC * NT   # total x tiles per core

F32 = mybir.dt.float32
BF16 = mybir.dt.bfloat16

AF = mybir.ActivationFunctionType


def build_nc(trace_sim=False):
    nc = bass.Bass()

    x_d = nc.dram_tensor("x", [BPC, SQ, DM], F32, kind="ExternalInput")
    ctx_d = nc.dram_tensor("context", [BPC, SKV, DC], F32, kind="ExternalInput")
    wq_d = nc.dram_tensor("Wq", [DM, INNER], F32, kind="ExternalInput")
    wk_d = nc.dram_tensor("Wk", [DC, INNER], F32, kind="ExternalInput")
    wv_d = nc.dram_tensor("Wv", [DC, INNER], F32, kind="ExternalInput")
    wo_d = nc.dram_tensor("Wout", [INNER, INNER], F32, kind="ExternalInput")
    bo_d = nc.dram_tensor("bout", [INNER], F32, kind="ExternalInput")
    out_d = nc.dram_tensor("out", [BPC, SQ, DM], F32, kind="ExternalOutput")

    with tile.TileContext(nc, trace_sim=trace_sim) as tc:
        with (
            tc.tile_pool(name="const", bufs=1) as consts,
            tc.tile_pool(name="kvp", bufs=2) as kvp,
            tc.tile_pool(name="xload", bufs=3) as xload,
            tc.tile_pool(name="xtp", bufs=3) as xtp,
            tc.tile_pool(name="qtp", bufs=2) as qtp,
            tc.tile_pool(name="etp", bufs=2) as etp,
            tc.tile_pool(name="rrp", bufs=8) as rrp,
            tc.tile_pool(name="anp", bufs=2) as anp,
            tc.tile_pool(name="atp", bufs=2) as atp,
            tc.tile_pool(name="osp", bufs=2) as osp,
            tc.tile_pool(name="pbig", bufs=3, space="PSUM") as pbig,
            tc.tile_pool(name="psc", bufs=2, space="PSUM") as psc,
            tc.tile_pool(name="pau", bufs=3, space="PSUM") as pau,
        ):
            # ---- weights: casting SWDGE loads, fp32 DRAM -> bf16 SBUF ----
            # layout "(c p) e -> p c e": partition p of chunk c holds row
            # c*128+p — same mapping as the DMA-transpose destination.
            # Emission order = DMA service order: what the kv phase and the
            # first tiles need comes first.
            wk_sb = consts.tile([128, DC // 128, INNER], BF16, tag="wk")
            nc.gpsimd.dma_start(out=wk_sb, in_=wk_d[:].rearrange("(c p) e -> p c e", p=128))
            wv_sb = consts.tile([128, DC // 128, INNER], BF16, tag="wv")
            nc.gpsimd.dma_start(out=wv_sb, in_=wv_d[:].rearrange("(c p) e -> p c e", p=128))
            wq_sb = consts.tile([128, DM // 128, INNER], BF16, tag="wq")
            wo_sb = consts.tile([128, INNER // 128, INNER], BF16, tag="wo")

            ones_row = consts.tile([1, 128], BF16, tag="ones_row")
            nc.vector.memset(ones_row, 1.0)
            bo_row = consts.tile([1, INNER], BF16, tag="bo_row")

            def emit_kv(b):
                # ctx cast-load into a 80-partition tile (pad rows 77..79 are
                # never read downstream; DMA-T needs p % 16 == 0)
                ctx_bf = kvp.tile([80, DC], BF16, tag="ctx")
                nc.gpsimd.dma_start(out=ctx_bf[0:SKV, :], in_=ctx_d[b])
                ctxT = kvp.tile([128, DC // 128, 80], BF16, tag="ctxT")
                nc.sync.dma_start_transpose(out=ctxT, in_=ctx_bf[:, :])

                # kT[e, kv]: lhsT = Wk chunk, rhs = ctxT chunk
                kT_sb = kvp.tile([128, INNER // 128, SKV], BF16, tag="kT")
                for i in range(INNER // 128):
                    pk = pbig.tile([128, 512], F32, tag="big")
                    for j in range(DC // 128):
                        nc.tensor.matmul(
                            out=pk[:, 0:SKV],
                            lhsT=wk_sb[:, j, i * 128:(i + 1) * 128],
                            rhs=ctxT[:, j, 0:SKV],
                            start=(j == 0), stop=(j == DC // 128 - 1),
                        )
                    nc.scalar.copy(out=kT_sb[:, i, :], in_=pk[:, 0:SKV])

                # v[kv, e] with an appended ones column per head:
                # v_aug[:, h, 0:64] = v_h, v_aug[:, h, 64] = 1 (denominator)
                pv = pbig.tile([128, 512], F32, tag="big")
                for j in range(DC // 128):
                    nc.tensor.matmul(
                        out=pv[0:SKV, :],
                        lhsT=ctxT[:, j, 0:SKV],
                        rhs=wv_sb[:, j, :],
                        start=(j == 0), stop=(j == DC // 128 - 1),
                    )
                v_aug = kvp.tile([SKV, H, 66], BF16, tag="v_aug")
                nc.scalar.copy(
                    out=v_aug[:, :, 0:64],
                    in_=pv[0:SKV, :].rearrange("p (h d) -> p h d", h=H),
                )
                nc.vector.memset(v_aug[:, :, 64:66], 1.0)
                return kT_sb, v_aug

            # per-tile stage emitters; state[k] carries live tiles of tile k
            def bs(k):
                return k // NT, (k % NT) * 512

            def emit_xload(k):
                b, s0 = bs(k)
                x_bf = xload.tile([128, 4, DM], BF16, tag="x")
                nc.gpsimd.dma_start(
                    out=x_bf,
                    in_=x_d[b, s0:s0 + 512, :].rearrange("(t p) d -> p t d", p=128),
                )
                return x_bf

            def emit_xT(x_bf):
                xT = xtp.tile([128, 4, 4, 128], BF16, tag="xT")
                for t in range(4):
                    nc.sync.dma_start_transpose(out=xT[:, t, :, :], in_=x_bf[:, t, :])
                return xT

            def emit_qT(xT):
                qT = qtp.tile([128, 4, 512], BF16, tag="qT")
                for i in range(4):
                    pq = pbig.tile([128, 512], F32, tag="big")
                    for j in range(4):
                        nc.tensor.matmul(
                            out=pq,
                            lhsT=wq_sb[:, j, i * 128:(i + 1) * 128],
                            rhs=xT[:, :, j, :],
                            start=(j == 0), stop=(j == 3),
                        )
                    nc.scalar.copy(out=qT[:, i, :], in_=pq)
                return qT

            def emit_scores(qT, kT_sb):
                et = etp.tile([SKV, H, 512], BF16, tag="et")
                for h in range(H):
                    i, r0 = h // 2, (h % 2) * 64
                    ps = psc.tile([SKV, 512], F32, tag="sc")
                    nc.tensor.matmul(
                        out=ps,
                        lhsT=kT_sb[r0:r0 + 64, i, :],
                        rhs=qT[r0:r0 + 64, i, :],
                        start=True, stop=True,
                    )
                    nc.scalar.activation(
                        out=et[:, h, :], in_=ps, func=AF.Exp, scale=0.125,
                    )
                return et

            def emit_attn(et, v_aug):
                attn_n = anp.tile([128, 4, 512], BF16, tag="attn_n")
                attnT = atp.tile([128, 4, 4, 128], BF16, tag="attnT")
                for c in range(4):
                    for g in range(2):
                        pa = pau.tile([128, 4, 66], F32, tag="attnU")
                        for hh in range(4):
                            h = g * 4 + hh
                            nc.tensor.matmul(
                                out=pa[:, hh, :],
                                lhsT=et[:, h, c * 128:(c + 1) * 128],
                                rhs=v_aug[:, h, :],
                                start=True, stop=True,
                            )
                        rr = rrp.tile([128, 4, 1], F32, tag="rr")
                        nc.vector.reciprocal(out=rr, in_=pa[:, :, 64:65])
                        out_ap = attn_n[:, c, g * 256:(g + 1) * 256].rearrange(
                            "p (h d) -> p h d", h=4
                        )
                        in0, in1 = broadcast_tensor_aps(pa[:, :, 0:64], rr)
                        nc.vector.tensor_mul(out_ap, in0, in1)
                    nc.sync.dma_start_transpose(
                        out=attnT[:, c, :, :], in_=attn_n[:, c, :],
                    )
                return attnT

            def emit_outproj(attnT, k):
                b, s0 = bs(k)
                osb = osp.tile([128, 4, 512], F32, tag="osb")
                for c in range(4):
                    po = pbig.tile([128, 512], F32, tag="big")
                    for j in range(4):
                        nc.tensor.matmul(
                            out=po,
                            lhsT=attnT[:, c, j, :],
                            rhs=wo_sb[:, j, :],
                            start=(j == 0), stop=False,
                        )
                    nc.tensor.matmul(
                        out=po, lhsT=ones_row, rhs=bo_row,
                        start=False, stop=True,
                    )
                    nc.vector.tensor_copy(osb[:, c, :], po)
                nc.gpsimd.dma_start(
                    out=out_d[b, s0:s0 + 512, :].rearrange("(t p) d -> p t d", p=128),
                    in_=osb,
                )

            # ---- software-pipelined main loop ----
            # st[k] = dict of live per-tile objects
            st = {}
            kv_of = {}  # tile index -> (kT_sb, v_aug)

            kv = emit_kv(0)
            # prologue: x(0), x(1) loads + xT(0)
            st[0] = {"x": emit_xload(0)}
            # load Wq now (after kv-phase inputs), Wout + bias a bit later
            nc.gpsimd.dma_start(out=wq_sb, in_=wq_d[:].rearrange("(c p) e -> p c e", p=128))
            st[0]["xT"] = emit_xT(st[0]["x"])
            st[1] = {"x": emit_xload(1)}
            nc.gpsimd.dma_start(out=wo_sb, in_=wo_d[:].rearrange("(c p) e -> p c e", p=128))
            nc.gpsimd.dma_start(out=bo_row, in_=bo_d[:].rearrange("(p e) -> p e", p=1))

            for k in range(NTILES + 2):
                # stage A: next-next x load
                if k + 2 < NTILES:
                    st[k + 2] = {"x": emit_xload(k + 2)}
                # stage B: next xT transpose
                if 0 < k + 1 < NTILES:
                    st[k + 1]["xT"] = emit_xT(st[k + 1]["x"])

                if k < NTILES:
                    b = k // NT
                    # prefetch next batch's k/v mid-batch so its PE work and
                    # ctx DMA land well before the batch boundary
                    if k % NT == NT - 3 and b + 1 < BPC:
                        kv_next = emit_kv(b + 1)
                    if k % NT == 0 and k > 0:
                        kv = kv_next
                    kv_of[k] = kv
                    # PE stage 1: qT(k)
                    st[k]["qT"] = emit_qT(st[k]["xT"])
                    # PE stage 2: scores+exp(k)
                    st[k]["et"] = emit_scores(st[k]["qT"], kv_of[k][0])
                # PE stage 3: attnU/norm/transpose(k-1)
                if 0 <= k - 1 < NTILES:
                    st[k - 1]["attnT"] = emit_attn(st[k - 1]["et"], kv_of[k - 1][1])
                # PE stage 4: outproj(k-2)
                if 0 <= k - 2:
                    emit_outproj(st[k - 2]["attnT"], k - 2)
                    del st[k - 2]

    # TRN2 hardware allows at most 1 semaphore wait per instruction; split
    # multi-wait instructions into standalone EventSemaphore waits.
    _bass_rust.generate_event_semaphores(nc)
    return nc


_NC_CACHE = None


def kernel(x, context, Wq, Wk, Wv, Wout, bout):
    global _NC_CACHE
    if _NC_CACHE is None:
        _NC_CACHE = build_nc()
    nc = _NC_CACHE

    f = lambda a: np.ascontiguousarray(np.asarray(a), dtype=np.float32)
    x, context = f(x), f(context)
    Wq, Wk, Wv, Wout, bout = f(Wq), f(Wk), f(Wv), f(Wout), f(bout)

    in_maps = [
        {
            "x": x[c * BPC:(c + 1) * BPC],
            "context": context[c * BPC:(c + 1) * BPC],
            "Wq": Wq, "Wk": Wk, "Wv": Wv, "Wout": Wout, "bout": bout,
        }
        for c in range(N_CORES)
    ]
    res = run_bass_kernel_spmd(nc, in_maps, core_ids=list(range(N_CORES)))
    return np.concatenate([r["out"] for r in res.results], axis=0)


# revision 12
# speedup vs baseline: 1.5801x; 1.0724x over previous
"""Cross-attention Trainium2 Bass kernel (bf16, software-pipelined).

Sharding: data-parallel over batch — 16 batches across 8 cores, 2 per core.
Weights replicated. Each core computes its 2 batches fully; no collectives.

All matmuls run in bf16 (1 cycle/row at any moving size on TRN2's PE).
PE transposes are eliminated entirely: x and attn are transposed by the
DMA crossbar (dma_start_transpose, 2-byte dtype, 16x128 xbar tiles), whose
destination mapping is out[p, c, f] = in[f, c*128 + p] (verified on hw).

Per 512-row x tile:
  x_bf   = bf16(x tile)                 (SWDGE casting DMA, Pool engine)
  xT     = DMA-transpose(x_bf)          -> [d, s] layout
  qT     = Wq^T @ xT                    (PE; PSUM->SBUF copy on ACT)
  per head h:  scT = kT_h^T @ qT_h      [77, 512] (PE)
               et_h = exp(0.125 * scT)  (ACT, bf16 out)
  per s-chunk c (128 rows), per 4-head group:
    pa[:, hh, :] = et_h_chunk^T @ [v_h | 1 | 1]   (PE; col 64 = softmax denom)
    rr = 1/pa[:, :, 64]                 (DVE)
    attn_n = pa[:, :, 0:64] * rr        (DVE, bf16, per-partition scalar)
  attnT  = DMA-transpose(attn_n)        -> [e, s] layout
  out    = attnT^T @ Wout + bout        (PE; bias as rank-1 ones matmul
           accumulated in PSUM; plain PSUM->SBUF copy on DVE; SWDGE store)

The per-engine instruction streams are software-pipelined with a 2-tile
lag so no engine waits on same-tile producers:
  iteration k (PE order): qT(k) | scores(k) | attnU(k-1) | outproj(k-2)
with x loads / DMA transposes issued 1-2 iterations ahead.

Weights are cast-loaded fp32->bf16 by SWDGE DMA, laid out
"(c p) e -> p c e" so partition p of chunk c holds row c*128+p, matching
the DMA-transpose output mapping. TRN2 allows 1 semaphore wait per
instruction — generate_event_semaphores() legalizes multi-wait
instructions that Tile emits.
"""

import numpy as np

import bass_rust as _bass_rust
import concourse.bass as bass
import concourse.mybir as mybir
import concourse.tile as tile
from concourse.bass import broadcast_tensor_aps
from concourse.bass_utils import run_bass_kernel_spmd

N_CORES = 8
B, SQ, DM = 16, 4096, 512
SKV, DC = 77, 768
H, DH = 8, 64
INNER = 512
BPC = B // N_CORES  # batches per core
NT = SQ // 512      # x tiles per batch
NTILES = BPC * NT   # total x tiles per core

F32 = mybir.dt.float32
BF16 = mybir.dt.bfloat16

AF = mybir.ActivationFunctionType


def build_nc(trace_sim=False):
    nc = bass.Bass()

    x_d = nc.dram_tensor("x", [BPC, SQ, DM], F32, kind="ExternalInput")
    ctx_d = nc.dram_tensor("context", [BPC, SKV, DC], F32, kind="ExternalInput")
    wq_d = nc.dram_tensor("Wq", [DM, INNER], F32, kind="ExternalInput")
    wk_d = nc.dram_tensor("Wk", [DC, INNER], F32, kind="ExternalInput")
    wv_d = nc.dram_tensor("Wv", [DC, INNER], F32, kind="ExternalInput")
    wo_d = nc.dram_tensor("Wout", [INNER, INNER], F32, kind="ExternalInput")
    bo_d = nc.dram_tensor("bout", [INNER], F32, kind="ExternalInput")
    out_d = nc.dram_tensor("out", [BPC, SQ, DM], F32, kind="ExternalOutput")

    with tile.TileContext(nc, trace_sim=trace_sim) as tc:
        with (
            tc.tile_pool(name="const", bufs=1) as consts,
            tc.tile_pool(name="kvp", bufs=2) as kvp,
            tc.tile_pool(name="xload", bufs=3) as xload,
            tc.tile_pool(name="xtp", bufs=3) as xtp,
            tc.tile_pool(name="qtp", bufs=2) as qtp,
            tc.tile_pool(name="etp", bufs=2) as etp,
            tc.tile_pool(name="rrp", bufs=8) as rrp,
            tc.tile_pool(name="anp", bufs=2) as anp,
            tc.tile_pool(name="atp", bufs=2) as atp,
            tc.tile_pool(name="osp", bufs=2) as osp,
            tc.tile_pool(name="pbig", bufs=3, space="PSUM") as pbig,
            tc.tile_pool(name="psc", bufs=2, space="PSUM") as psc,
            tc.tile_pool(name="pau", bufs=3, space="PSUM") as pau,
        ):
            # ---- weights: casting SWDGE loads, fp32 DRAM -> bf16 SBUF ----
            # layout "(c p) e -> p c e": partition p of chunk c holds row
            # c*128+p — same mapping as the DMA-transpose destination.
            # Declared here; loads are emitted below in DMA service order so
            # what the kv phase and the first tiles need lands first.
            wk_sb = consts.tile([128, DC // 128, INNER], BF16, tag="wk")
            wv_sb = consts.tile([128, DC // 128, INNER], BF16, tag="wv")
            wq_sb = consts.tile([128, DM // 128, INNER], BF16, tag="wq")
            wo_sb = consts.tile([128, INNER // 128, INNER], BF16, tag="wo")
            bias_b = consts.tile([128, INNER], F32, tag="bias")

            def emit_ctx(b):
                # ctx cast-load into a 80-partition tile (pad rows 77..79 are
                # never read downstream; DMA-T needs p % 16 == 0)
                ctx_bf = kvp.tile([80, DC], BF16, tag="ctx")
                nc.gpsimd.dma_start(out=ctx_bf[0:SKV, :], in_=ctx_d[b])
                ctxT = kvp.tile([128, DC // 128, 80], BF16, tag="ctxT")
                nc.sync.dma_start_transpose(out=ctxT, in_=ctx_bf[:, :])
                return ctxT

            def emit_kv(ctxT):
                # kT[e, kv]: lhsT = Wk chunk, rhs = ctxT chunk
                kT_sb = kvp.tile([128, INNER // 128, SKV], BF16, tag="kT")
                for i in range(INNER // 128):
                    pk = pbig.tile([128, 512], F32, tag="big")
                    for j in range(DC // 128):
                        nc.tensor.matmul(
                            out=pk[:, 0:SKV],
                            lhsT=wk_sb[:, j, i * 128:(i + 1) * 128],
                            rhs=ctxT[:, j, 0:SKV],
                            start=(j == 0), stop=(j == DC // 128 - 1),
                        )
                    nc.scalar.copy(out=kT_sb[:, i, :], in_=pk[:, 0:SKV])

                # v[kv, e] with an appended ones column per head:
                # v_aug[:, h, 0:64] = v_h, v_aug[:, h, 64] = 1 (denominator)
                pv = pbig.tile([128, 512], F32, tag="big")
                for j in range(DC // 128):
                    nc.tensor.matmul(
                        out=pv[0:SKV, :],
                        lhsT=ctxT[:, j, 0:SKV],
                        rhs=wv_sb[:, j, :],
                        start=(j == 0), stop=(j == DC // 128 - 1),
                    )
                v_aug = kvp.tile([SKV, H, 66], BF16, tag="v_aug")
                nc.scalar.copy(
                    out=v_aug[:, :, 0:64],
                    in_=pv[0:SKV, :].rearrange("p (h d) -> p h d", h=H),
                )
                nc.vector.memset(v_aug[:, :, 64:66], 1.0)
                return kT_sb, v_aug

            # per-tile stage emitters; state[k] carries live tiles of tile k
            def bs(k):
                return k // NT, (k % NT) * 512

            def emit_xload(k):
                b, s0 = bs(k)
                x_bf = xload.tile([128, 4, DM], BF16, tag="x")
                nc.gpsimd.dma_start(
                    out=x_bf,
                    in_=x_d[b, s0:s0 + 512, :].rearrange("(t p) d -> p t d", p=128),
                )
                return x_bf

            def emit_xT(x_bf):
                xT = xtp.tile([128, 4, 4, 128], BF16, tag="xT")
                for t in range(4):
                    nc.sync.dma_start_transpose(out=xT[:, t, :, :], in_=x_bf[:, t, :])
                return xT

            def emit_qT(xT):
                qT = qtp.tile([128, 4, 512], BF16, tag="qT")
                for i in range(4):
                    pq = pbig.tile([128, 512], F32, tag="big")
                    for j in range(4):
                        nc.tensor.matmul(
                            out=pq,
                            lhsT=wq_sb[:, j, i * 128:(i + 1) * 128],
                            rhs=xT[:, :, j, :],
                            start=(j == 0), stop=(j == 3),
                        )
                    nc.scalar.copy(out=qT[:, i, :], in_=pq)
                return qT

            def emit_scores(qT, kT_sb):
                et = etp.tile([SKV, H, 512], BF16, tag="et")
                for h in range(H):
                    i, r0 = h // 2, (h % 2) * 64
                    ps = psc.tile([SKV, 512], F32, tag="sc")
                    nc.tensor.matmul(
                        out=ps,
                        lhsT=kT_sb[r0:r0 + 64, i, :],
                        rhs=qT[r0:r0 + 64, i, :],
                        start=True, stop=True,
                    )
                    nc.scalar.activation(
                        out=et[:, h, :], in_=ps, func=AF.Exp, scale=0.125,
                    )
                return et

            def emit_attn(et, v_aug):
                attn_n = anp.tile([128, 4, 512], BF16, tag="attn_n")
                attnT = atp.tile([128, 4, 4, 128], BF16, tag="attnT")
                for c in range(4):
                    for g in range(2):
                        pa = pau.tile([128, 4, 66], F32, tag="attnU")
                        for hh in range(4):
                            h = g * 4 + hh
                            nc.tensor.matmul(
                                out=pa[:, hh, :],
                                lhsT=et[:, h, c * 128:(c + 1) * 128],
                                rhs=v_aug[:, h, :],
                                start=True, stop=True,
                            )
                        rr = rrp.tile([128, 4, 1], F32, tag="rr")
                        nc.vector.reciprocal(out=rr, in_=pa[:, :, 64:65])
                        out_ap = attn_n[:, c, g * 256:(g + 1) * 256].rearrange(
                            "p (h d) -> p h d", h=4
                        )
                        in0, in1 = broadcast_tensor_aps(pa[:, :, 0:64], rr)
                        nc.vector.tensor_mul(out_ap, in0, in1)
                    nc.sync.dma_start_transpose(
                        out=attnT[:, c, :, :], in_=attn_n[:, c, :],
                    )
                return attnT

            def emit_outproj(attnT, k):
                b, s0 = bs(k)
                osb = osp.tile([128, 4, 512], F32, tag="osb")
                for c in range(4):
                    po = pbig.tile([128, 512], F32, tag="big")
                    for j in range(4):
                        nc.tensor.matmul(
                            out=po,
                            lhsT=attnT[:, c, j, :],
                            rhs=wo_sb[:, j, :],
                            start=(j == 0), stop=(j == 3),
                        )
                    nc.vector.tensor_add(osb[:, c, :], po, bias_b)
                nc.gpsimd.dma_start(
                    out=out_d[b, s0:s0 + 512, :].rearrange("(t p) d -> p t d", p=128),
                    in_=osb,
                )

            # ---- software-pipelined main loop ----
            # st[k] = dict of live per-tile objects
            st = {}
            kv_of = {}  # tile index -> (kT_sb, v_aug)

            # prologue, in DMA service order: ctx(0) (tiny, kv-critical),
            # then Wk/Wv (kv matmuls), then x(0)/Wq (first qT), then the rest
            ctxT0 = emit_ctx(0)
            nc.gpsimd.dma_start(out=wk_sb, in_=wk_d[:].rearrange("(c p) e -> p c e", p=128))
            nc.gpsimd.dma_start(out=wv_sb, in_=wv_d[:].rearrange("(c p) e -> p c e", p=128))
            kv = emit_kv(ctxT0)
            st[0] = {"x": emit_xload(0)}
            nc.gpsimd.dma_start(out=wq_sb, in_=wq_d[:].rearrange("(c p) e -> p c e", p=128))
            st[0]["xT"] = emit_xT(st[0]["x"])
            st[1] = {"x": emit_xload(1)}
            nc.gpsimd.dma_start(out=wo_sb, in_=wo_d[:].rearrange("(c p) e -> p c e", p=128))
            nc.gpsimd.dma_start(out=bias_b, in_=bo_d[:].partition_broadcast(128))

            for k in range(NTILES + 2):
                # stage A: next-next x load
                if k + 2 < NTILES:
                    st[k + 2] = {"x": emit_xload(k + 2)}
                # stage B: next xT transpose
                if 0 < k + 1 < NTILES:
                    st[k + 1]["xT"] = emit_xT(st[k + 1]["x"])

                if k < NTILES:
                    b = k // NT
                    # prefetch next batch's ctx early and its k/v mid-batch so
                    # the PE work and ctx DMA land before the batch boundary
                    if k % NT == 2 and b + 1 < BPC:
                        ctxT_next = emit_ctx(b + 1)
                    if k % NT == NT - 3 and b + 1 < BPC:
                        kv_next = emit_kv(ctxT_next)
                    if k % NT == 0 and k > 0:
                        kv = kv_next
                    kv_of[k] = kv
                    # PE stage 1: qT(k)
                    st[k]["qT"] = emit_qT(st[k]["xT"])
                    # PE stage 2: scores+exp(k)
                    st[k]["et"] = emit_scores(st[k]["qT"], kv_of[k][0])
                # PE stage 3: attnU/norm/transpose(k-1)
                if 0 <= k - 1 < NTILES:
                    st[k - 1]["attnT"] = emit_attn(st[k - 1]["et"], kv_of[k - 1][1])
                # PE stage 4: outproj(k-2)
                if 0 <= k - 2:
                    emit_outproj(st[k - 2]["attnT"], k - 2)
                    del st[k - 2]

    # TRN2 hardware allows at most 1 semaphore wait per instruction; split
    # multi-wait instructions into standalone EventSemaphore waits.
    _bass_rust.generate_event_semaphores(nc)
    return nc


_NC_CACHE = None


def kernel(x, context, Wq, Wk, Wv, Wout, bout):
    global _NC_CACHE
    if _NC_CACHE is None:
        _NC_CACHE = build_nc()
    nc = _NC_CACHE

    f = lambda a: np.ascontiguousarray(np.asarray(a), dtype=np.float32)
    x, context = f(x), f(context)
    Wq, Wk, Wv, Wout, bout = f(Wq), f(Wk), f(Wv), f(Wout), f(bout)

    in_maps = [
        {
            "x": x[c * BPC:(c + 1) * BPC],
            "context": context[c * BPC:(c + 1) * BPC],
            "Wq": Wq, "Wk": Wk, "Wv": Wv, "Wout": Wout, "bout": bout,
        }
        for c in range(N_CORES)
    ]
    res = run_bass_kernel_spmd(nc, in_maps, core_ids=list(range(N_CORES)))
    return np.concatenate([r["out"] for r in res.results], axis=0)


# revision 15
# speedup vs baseline: 1.5814x; 1.0009x over previous
"""Cross-attention Trainium2 Bass kernel (bf16, software-pipelined).

Sharding: data-parallel over batch — 16 batches across 8 cores, 2 per core.
Weights replicated. Each core computes its 2 batches fully; no collectives.

All matmuls run in bf16 (1 cycle/row at any moving size on TRN2's PE).
PE transposes are eliminated entirely: x and attn are transposed by the
DMA crossbar (dma_start_transpose, 2-byte dtype, 16x128 xbar tiles), whose
destination mapping is out[p, c, f] = in[f, c*128 + p] (verified on hw).

Per 512-row x tile:
  x_bf   = bf16(x tile)                 (SWDGE casting DMA, Pool engine)
  xT     = DMA-transpose(x_bf)          -> [d, s] layout
  qT     = Wq^T @ xT                    (PE; PSUM->SBUF copy on ACT)
  per head h:  scT = kT_h^T @ qT_h      [77, 512] (PE)
               et_h = exp(0.125 * scT)  (ACT, bf16 out)
  per s-chunk c (128 rows), per 4-head group:
    pa[:, hh, :] = et_h_chunk^T @ [v_h | 1 | 1]   (PE; col 64 = softmax denom)
    rr = 1/pa[:, :, 64]                 (DVE)
    attn_n = pa[:, :, 0:64] * rr        (DVE, bf16, per-partition scalar)
  attnT  = DMA-transpose(attn_n)        -> [e, s] layout
  out    = attnT^T @ Wout + bout        (PE; bias as rank-1 ones matmul
           accumulated in PSUM; plain PSUM->SBUF copy on DVE; SWDGE store)

The per-engine instruction streams are software-pipelined with a 2-tile
lag so no engine waits on same-tile producers:
  iteration k (PE order): qT(k) | scores(k) | attnU(k-1) | outproj(k-2)
with x loads / DMA transposes issued 1-2 iterations ahead.

Weights are cast-loaded fp32->bf16 by SWDGE DMA, laid out
"(c p) e -> p c e" so partition p of chunk c holds row c*128+p, matching
the DMA-transpose output mapping. TRN2 allows 1 semaphore wait per
instruction — generate_event_semaphores() legalizes multi-wait
instructions that Tile emits.
"""

import numpy as np

import bass_rust as _bass_rust
import concourse.bass as bass
import concourse.mybir as mybir
import concourse.tile as tile
from concourse.bass import broadcast_tensor_aps
from concourse.bass_utils import run_bass_kernel_spmd

N_CORES = 8
B, SQ, DM = 16, 4096, 512
SKV, DC = 77, 768
H, DH = 8, 64
INNER = 512
BPC = B // N_CORES  # batches per core
NT = SQ // 512      # x tiles per batch
NTILES = BPC * NT   # total x tiles per core

F32 = mybir.dt.float32
BF16 = mybir.dt.bfloat16

AF = mybir.ActivationFunctionType


def build_nc(trace_sim=False):
    nc = bass.Bass()

    x_d = nc.dram_tensor("x", [BPC, SQ, DM], F32, kind="ExternalInput")
    ctx_d = nc.dram_tensor("context", [BPC, SKV, DC], F32, kind="ExternalInput")
    wq_d = nc.dram_tensor("Wq", [DM, INNER], F32, kind="ExternalInput")
    wk_d = nc.dram_tensor("Wk", [DC, INNER], F32, kind="ExternalInput")
    wv_d = nc.dram_tensor("Wv", [DC, INNER], F32, kind="ExternalInput")
    wo_d = nc.dram_tensor("Wout", [INNER, INNER], F32, kind="ExternalInput")
    bo_d = nc.dram_tensor("bout", [INNER], F32, kind="ExternalInput")
    out_d = nc.dram_tensor("out", [BPC, SQ, DM], F32, kind="ExternalOutput")

    with tile.TileContext(nc, trace_sim=trace_sim) as tc:
        with (
            tc.tile_pool(name="const", bufs=1) as consts,
            tc.tile_pool(name="kvp", bufs=2) as kvp,
            tc.tile_pool(name="xload", bufs=3) as xload,
            tc.tile_pool(name="xtp", bufs=3) as xtp,
            tc.tile_pool(name="qtp", bufs=2) as qtp,
            tc.tile_pool(name="etp", bufs=2) as etp,
            tc.tile_pool(name="rrp", bufs=8) as rrp,
            tc.tile_pool(name="anp", bufs=2) as anp,
            tc.tile_pool(name="atp", bufs=2) as atp,
            tc.tile_pool(name="osp", bufs=2) as osp,
            tc.tile_pool(name="pbig", bufs=3, space="PSUM") as pbig,
            tc.tile_pool(name="psc", bufs=2, space="PSUM") as psc,
            tc.tile_pool(name="pau", bufs=3, space="PSUM") as pau,
        ):
            # ---- weights: casting SWDGE loads, fp32 DRAM -> bf16 SBUF ----
            # layout "(c p) e -> p c e": partition p of chunk c holds row
            # c*128+p — same mapping as the DMA-transpose destination.
            # Declared here; loads are emitted below in DMA service order so
            # what the kv phase and the first tiles need lands first.
            wk_sb = consts.tile([128, DC // 128, INNER], BF16, tag="wk")
            wv_sb = consts.tile([128, DC // 128, INNER], BF16, tag="wv")
            wq_sb = consts.tile([128, DM // 128, INNER], BF16, tag="wq")
            wo_sb = consts.tile([128, INNER // 128, INNER], BF16, tag="wo")
            bias_b = consts.tile([128, INNER], F32, tag="bias")

            def emit_ctx(b):
                # ctx cast-load into a 80-partition tile (pad rows 77..79 are
                # never read downstream; DMA-T needs p % 16 == 0)
                ctx_bf = kvp.tile([80, DC], BF16, tag="ctx")
                nc.gpsimd.dma_start(out=ctx_bf[0:SKV, :], in_=ctx_d[b])
                ctxT = kvp.tile([128, DC // 128, 80], BF16, tag="ctxT")
                nc.sync.dma_start_transpose(out=ctxT, in_=ctx_bf[:, :])
                return ctxT

            def emit_kv(ctxT):
                # kT[e, kv]: lhsT = Wk chunk, rhs = ctxT chunk
                kT_sb = kvp.tile([128, INNER // 128, SKV], BF16, tag="kT")
                for i in range(INNER // 128):
                    pk = pbig.tile([128, 512], F32, tag="big")
                    for j in range(DC // 128):
                        nc.tensor.matmul(
                            out=pk[:, 0:SKV],
                            lhsT=wk_sb[:, j, i * 128:(i + 1) * 128],
                            rhs=ctxT[:, j, 0:SKV],
                            start=(j == 0), stop=(j == DC // 128 - 1),
                        )
                    nc.scalar.copy(out=kT_sb[:, i, :], in_=pk[:, 0:SKV])

                # v[kv, e] with an appended ones column per head:
                # v_aug[:, h, 0:64] = v_h, v_aug[:, h, 64] = 1 (denominator)
                pv = pbig.tile([128, 512], F32, tag="big")
                for j in range(DC // 128):
                    nc.tensor.matmul(
                        out=pv[0:SKV, :],
                        lhsT=ctxT[:, j, 0:SKV],
                        rhs=wv_sb[:, j, :],
                        start=(j == 0), stop=(j == DC // 128 - 1),
                    )
                v_aug = kvp.tile([SKV, H, 66], BF16, tag="v_aug")
                nc.scalar.copy(
                    out=v_aug[:, :, 0:64],
                    in_=pv[0:SKV, :].rearrange("p (h d) -> p h d", h=H),
                )
                nc.vector.memset(v_aug[:, :, 64:66], 1.0)
                return kT_sb, v_aug

            # per-tile stage emitters; state[k] carries live tiles of tile k
            def bs(k):
                return k // NT, (k % NT) * 512

            def emit_xload(k):
                b, s0 = bs(k)
                x_bf = xload.tile([128, 4, DM], BF16, tag="x")
                nc.gpsimd.dma_start(
                    out=x_bf,
                    in_=x_d[b, s0:s0 + 512, :].rearrange("(t p) d -> p t d", p=128),
                )
                return x_bf

            def emit_xT(x_bf):
                xT = xtp.tile([128, 4, 4, 128], BF16, tag="xT")
                for t in range(4):
                    nc.sync.dma_start_transpose(out=xT[:, t, :, :], in_=x_bf[:, t, :])
                return xT

            def emit_qT(xT):
                qT = qtp.tile([128, 4, 512], BF16, tag="qT")
                for i in range(4):
                    pq = pbig.tile([128, 512], F32, tag="big")
                    for j in range(4):
                        nc.tensor.matmul(
                            out=pq,
                            lhsT=wq_sb[:, j, i * 128:(i + 1) * 128],
                            rhs=xT[:, :, j, :],
                            start=(j == 0), stop=(j == 3),
                        )
                    nc.scalar.copy(out=qT[:, i, :], in_=pq)
                return qT

            def emit_scores(qT, kT_sb):
                et = etp.tile([SKV, H, 512], BF16, tag="et")
                for h in range(H):
                    i, r0 = h // 2, (h % 2) * 64
                    ps = psc.tile([SKV, 512], F32, tag="sc")
                    nc.tensor.matmul(
                        out=ps,
                        lhsT=kT_sb[r0:r0 + 64, i, :],
                        rhs=qT[r0:r0 + 64, i, :],
                        start=True, stop=True,
                    )
                    nc.scalar.activation(
                        out=et[:, h, :], in_=ps, func=AF.Exp, scale=0.125,
                    )
                return et

            def emit_attn(et, v_aug):
                attn_n = anp.tile([128, 4, 512], BF16, tag="attn_n")
                attnT = atp.tile([128, 4, 4, 128], BF16, tag="attnT")
                for c in range(4):
                    for g in range(2):
                        pa = pau.tile([128, 4, 66], F32, tag="attnU")
                        for hh in range(4):
                            h = g * 4 + hh
                            nc.tensor.matmul(
                                out=pa[:, hh, :],
                                lhsT=et[:, h, c * 128:(c + 1) * 128],
                                rhs=v_aug[:, h, :],
                                start=True, stop=True,
                            )
                        rr = rrp.tile([128, 4, 1], F32, tag="rr")
                        nc.vector.reciprocal(out=rr, in_=pa[:, :, 64:65])
                        out_ap = attn_n[:, c, g * 256:(g + 1) * 256].rearrange(
                            "p (h d) -> p h d", h=4
                        )
                        in0, in1 = broadcast_tensor_aps(pa[:, :, 0:64], rr)
                        nc.vector.tensor_mul(out_ap, in0, in1)
                    nc.sync.dma_start_transpose(
                        out=attnT[:, c, :, :], in_=attn_n[:, c, :],
                    )
                return attnT

            def emit_outproj(attnT, k):
                b, s0 = bs(k)
                osb = osp.tile([128, 4, 512], F32, tag="osb")
                for c in range(4):
                    po = pbig.tile([128, 512], F32, tag="big")
                    for j in range(4):
                        nc.tensor.matmul(
                            out=po,
                            lhsT=attnT[:, c, j, :],
                            rhs=wo_sb[:, j, :],
                            start=(j == 0), stop=(j == 3),
                        )
                    nc.vector.tensor_add(osb[:, c, :], po, bias_b)
                nc.gpsimd.dma_start(
                    out=out_d[b, s0:s0 + 512, :].rearrange("(t p) d -> p t d", p=128),
                    in_=osb,
                )

            # ---- software-pipelined main loop ----
            # st[k] = dict of live per-tile objects
            st = {}
            kv_of = {}  # tile index -> (kT_sb, v_aug)

            # prologue, in DMA service order: ctx(0) (tiny, kv-critical),
            # then Wk/Wv (kv matmuls), then x(0)/Wq (first qT), then the rest
            ctxT0 = emit_ctx(0)
            st[0] = {"x": emit_xload(0)}
            nc.gpsimd.dma_start(out=wq_sb, in_=wq_d[:].rearrange("(c p) e -> p c e", p=128))
            st[0]["xT"] = emit_xT(st[0]["x"])
            nc.gpsimd.dma_start(out=wk_sb, in_=wk_d[:].rearrange("(c p) e -> p c e", p=128))
            nc.gpsimd.dma_start(out=wv_sb, in_=wv_d[:].rearrange("(c p) e -> p c e", p=128))
            kv = emit_kv(ctxT0)
            st[1] = {"x": emit_xload(1)}
            nc.gpsimd.dma_start(out=wo_sb, in_=wo_d[:].rearrange("(c p) e -> p c e", p=128))
            nc.gpsimd.dma_start(out=bias_b, in_=bo_d[:].partition_broadcast(128))

            for k in range(NTILES + 2):
                # stage A: next-next x load
                if k + 2 < NTILES:
                    st[k + 2] = {"x": emit_xload(k + 2)}
                # stage B: next xT transpose
                if 0 < k + 1 < NTILES:
                    st[k + 1]["xT"] = emit_xT(st[k + 1]["x"])

                if k < NTILES:
                    b = k // NT
                    # prefetch next batch's ctx early and its k/v mid-batch so
                    # the PE work and ctx DMA land before the batch boundary
                    if k % NT == 2 and b + 1 < BPC:
                        ctxT_next = emit_ctx(b + 1)
                    if k % NT == NT - 3 and b + 1 < BPC:
                        kv_next = emit_kv(ctxT_next)
                    if k % NT == 0 and k > 0:
                        kv = kv_next
                    kv_of[k] = kv
                    # PE stage 1: qT(k)
                    st[k]["qT"] = emit_qT(st[k]["xT"])
                    # PE stage 2: scores+exp(k)
                    st[k]["et"] = emit_scores(st[k]["qT"], kv_of[k][0])
                # PE stage 3: attnU/norm/transpose(k-1)
                if 0 <= k - 1 < NTILES and "attnT" not in st[k - 1]:
                    st[k - 1]["attnT"] = emit_attn(st[k - 1]["et"], kv_of[k - 1][1])
                # PE stage 4: outproj(k-2)
                if 0 <= k - 2:
                    emit_outproj(st[k - 2]["attnT"], k - 2)
                    del st[k - 2]
                # epilogue shortcut: run the last tile's attention stage
                # lag-0 (its exps are long done by this point in the PE
                # stream) so the final outproj isn't stuck behind a fresh
                # DMA transpose at drain time
                if k == NTILES - 1:
                    st[k]["attnT"] = emit_attn(st[k]["et"], kv_of[k][1])

    # TRN2 hardware allows at most 1 semaphore wait per instruction; split
    # multi-wait instructions into standalone EventSemaphore waits.
    _bass_rust.generate_event_semaphores(nc)
    return nc


_NC_CACHE = None


def kernel(x, context, Wq, Wk, Wv, Wout, bout):
    global _NC_CACHE
    if _NC_CACHE is None:
        _NC_CACHE = build_nc()
    nc = _NC_CACHE

    f = lambda a: np.ascontiguousarray(np.asarray(a), dtype=np.float32)
    x, context = f(x), f(context)
    Wq, Wk, Wv, Wout, bout = f(Wq), f(Wk), f(Wv), f(Wout), f(bout)

    in_maps = [
        {
            "x": x[c * BPC:(c + 1) * BPC],
            "context": context[c * BPC:(c + 1) * BPC],
            "Wq": Wq, "Wk": Wk, "Wv": Wv, "Wout": Wout, "bout": bout,
        }
        for c in range(N_CORES)
    ]
    res = run_bass_kernel_spmd(nc, in_maps, core_ids=list(range(N_CORES)))
    return np.concatenate([r["out"] for r in res.results], axis=0)


# revision 18
# speedup vs baseline: 1.5873x; 1.0037x over previous
"""Cross-attention Trainium2 Bass kernel (bf16, software-pipelined).

Sharding: data-parallel over batch — 16 batches across 8 cores, 2 per core.
Weights replicated. Each core computes its 2 batches fully; no collectives.

All matmuls run in bf16 (1 cycle/row at any moving size on TRN2's PE).
PE transposes are eliminated entirely: x and attn are transposed by the
DMA crossbar (dma_start_transpose, 2-byte dtype, 16x128 xbar tiles), whose
destination mapping is out[p, c, f] = in[f, c*128 + p] (verified on hw).

Per 512-row x tile:
  x_bf   = bf16(x tile)                 (SWDGE casting DMA, Pool engine)
  xT     = DMA-transpose(x_bf)          -> [d, s] layout
  qT     = Wq^T @ xT                    (PE; PSUM->SBUF copy on ACT)
  per head h:  scT = kT_h^T @ qT_h      [77, 512] (PE)
               et_h = exp(0.125 * scT)  (ACT, bf16 out)
  per s-chunk c (128 rows), per 4-head group:
    pa[:, hh, :] = et_h_chunk^T @ [v_h | 1 | 1]   (PE; col 64 = softmax denom)
    rr = 1/pa[:, :, 64]                 (DVE)
    attn_n = pa[:, :, 0:64] * rr        (DVE, bf16, per-partition scalar)
  attnT  = DMA-transpose(attn_n)        -> [e, s] layout
  out    = attnT^T @ Wout + bout        (PE; bias as rank-1 ones matmul
           accumulated in PSUM; plain PSUM->SBUF copy on DVE; SWDGE store)

The per-engine instruction streams are software-pipelined with a 2-tile
lag so no engine waits on same-tile producers:
  iteration k (PE order): qT(k) | scores(k) | attnU(k-1) | outproj(k-2)
with x loads / DMA transposes issued 1-2 iterations ahead.

Weights are cast-loaded fp32->bf16 by SWDGE DMA, laid out
"(c p) e -> p c e" so partition p of chunk c holds row c*128+p, matching
the DMA-transpose output mapping. TRN2 allows 1 semaphore wait per
instruction — generate_event_semaphores() legalizes multi-wait
instructions that Tile emits.
"""

import numpy as np

import bass_rust as _bass_rust
import concourse.bass as bass
import concourse.mybir as mybir
import concourse.tile as tile
from concourse.bass import broadcast_tensor_aps
from concourse.bass_utils import run_bass_kernel_spmd

N_CORES = 8
B, SQ, DM = 16, 4096, 512
SKV, DC = 77, 768
H, DH = 8, 64
INNER = 512
BPC = B // N_CORES  # batches per core
NT = SQ // 512      # x tiles per batch
NTILES = BPC * NT   # total x tiles per core

F32 = mybir.dt.float32
BF16 = mybir.dt.bfloat16

AF = mybir.ActivationFunctionType


def build_nc(trace_sim=False):
    nc = bass.Bass()

    x_d = nc.dram_tensor("x", [BPC, SQ, DM], F32, kind="ExternalInput")
    ctx_d = nc.dram_tensor("context", [BPC, SKV, DC], F32, kind="ExternalInput")
    wq_d = nc.dram_tensor("Wq", [DM, INNER], F32, kind="ExternalInput")
    wk_d = nc.dram_tensor("Wk", [DC, INNER], F32, kind="ExternalInput")
    wv_d = nc.dram_tensor("Wv", [DC, INNER], F32, kind="ExternalInput")
    wo_d = nc.dram_tensor("Wout", [INNER, INNER], F32, kind="ExternalInput")
    bo_d = nc.dram_tensor("bout", [INNER], F32, kind="ExternalInput")
    out_d = nc.dram_tensor("out", [BPC, SQ, DM], F32, kind="ExternalOutput")

    with tile.TileContext(nc, trace_sim=trace_sim) as tc:
        with (
            tc.tile_pool(name="const", bufs=1) as consts,
            tc.tile_pool(name="kvp", bufs=2) as kvp,
            tc.tile_pool(name="xload", bufs=3) as xload,
            tc.tile_pool(name="xtp", bufs=3) as xtp,
            tc.tile_pool(name="qtp", bufs=2) as qtp,
            tc.tile_pool(name="etp", bufs=2) as etp,
            tc.tile_pool(name="rrp", bufs=8) as rrp,
            tc.tile_pool(name="anp", bufs=2) as anp,
            tc.tile_pool(name="atp", bufs=2) as atp,
            tc.tile_pool(name="osp", bufs=2) as osp,
            tc.tile_pool(name="pbig", bufs=3, space="PSUM") as pbig,
            tc.tile_pool(name="psc", bufs=2, space="PSUM") as psc,
            tc.tile_pool(name="pau", bufs=3, space="PSUM") as pau,
        ):
            # ---- weights: casting SWDGE loads, fp32 DRAM -> bf16 SBUF ----
            # layout "(c p) e -> p c e": partition p of chunk c holds row
            # c*128+p — same mapping as the DMA-transpose destination.
            # Declared here; loads are emitted below in DMA service order so
            # what the kv phase and the first tiles need lands first.
            wk_sb = consts.tile([128, DC // 128, INNER], BF16, tag="wk")
            wv_sb = consts.tile([128, DC // 128, INNER], BF16, tag="wv")
            wq_sb = consts.tile([128, DM // 128, INNER], BF16, tag="wq")
            wo_sb = consts.tile([128, INNER // 128, INNER], BF16, tag="wo")
            bias_b = consts.tile([128, INNER], F32, tag="bias")

            def emit_ctx(b):
                # ctx cast-load into a 80-partition tile (pad rows 77..79 are
                # never read downstream; DMA-T needs p % 16 == 0)
                ctx_bf = kvp.tile([80, DC], BF16, tag="ctx")
                nc.gpsimd.dma_start(out=ctx_bf[0:SKV, :], in_=ctx_d[b])
                ctxT = kvp.tile([128, DC // 128, 80], BF16, tag="ctxT")
                nc.sync.dma_start_transpose(out=ctxT, in_=ctx_bf[:, :])
                return ctxT

            def emit_kv(ctxT):
                # kT[e, kv]: lhsT = Wk chunk, rhs = ctxT chunk
                kT_sb = kvp.tile([128, INNER // 128, SKV], BF16, tag="kT")
                for i in range(INNER // 128):
                    pk = pbig.tile([128, 512], F32, tag="big")
                    for j in range(DC // 128):
                        nc.tensor.matmul(
                            out=pk[:, 0:SKV],
                            lhsT=wk_sb[:, j, i * 128:(i + 1) * 128],
                            rhs=ctxT[:, j, 0:SKV],
                            start=(j == 0), stop=(j == DC // 128 - 1),
                        )
                    nc.scalar.copy(out=kT_sb[:, i, :], in_=pk[:, 0:SKV])

                # v[kv, e] with an appended ones column per head:
                # v_aug[:, h, 0:64] = v_h, v_aug[:, h, 64] = 1 (denominator)
                pv = pbig.tile([128, 512], F32, tag="big")
                for j in range(DC // 128):
                    nc.tensor.matmul(
                        out=pv[0:SKV, :],
                        lhsT=ctxT[:, j, 0:SKV],
                        rhs=wv_sb[:, j, :],
                        start=(j == 0), stop=(j == DC // 128 - 1),
                    )
                v_aug = kvp.tile([SKV, H, 66], BF16, tag="v_aug")
                nc.scalar.copy(
                    out=v_aug[:, :, 0:64],
                    in_=pv[0:SKV, :].rearrange("p (h d) -> p h d", h=H),
                )
                nc.vector.memset(v_aug[:, :, 64:66], 1.0)
                return kT_sb, v_aug

            # per-tile stage emitters; state[k] carries live tiles of tile k
            def bs(k):
                return k // NT, (k % NT) * 512

            def emit_xload(k):
                b, s0 = bs(k)
                x_bf = xload.tile([128, 4, DM], BF16, tag="x")
                nc.gpsimd.dma_start(
                    out=x_bf,
                    in_=x_d[b, s0:s0 + 512, :].rearrange("(t p) d -> p t d", p=128),
                )
                return x_bf

            def emit_xT(x_bf):
                xT = xtp.tile([128, 4, 4, 128], BF16, tag="xT")
                for t in range(4):
                    nc.sync.dma_start_transpose(out=xT[:, t, :, :], in_=x_bf[:, t, :])
                return xT

            def emit_qT(xT):
                qT = qtp.tile([128, 4, 512], BF16, tag="qT")
                for i in range(4):
                    pq = pbig.tile([128, 512], F32, tag="big")
                    for j in range(4):
                        nc.tensor.matmul(
                            out=pq,
                            lhsT=wq_sb[:, j, i * 128:(i + 1) * 128],
                            rhs=xT[:, :, j, :],
                            start=(j == 0), stop=(j == 3),
                        )
                    nc.scalar.copy(out=qT[:, i, :], in_=pq)
                return qT

            def emit_scores(qT, kT_sb):
                et = etp.tile([SKV, H, 512], BF16, tag="et")
                for h in range(H):
                    i, r0 = h // 2, (h % 2) * 64
                    ps = psc.tile([SKV, 512], F32, tag="sc")
                    nc.tensor.matmul(
                        out=ps,
                        lhsT=kT_sb[r0:r0 + 64, i, :],
                        rhs=qT[r0:r0 + 64, i, :],
                        start=True, stop=True,
                    )
                    nc.scalar.activation(
                        out=et[:, h, :], in_=ps, func=AF.Exp, scale=0.125,
                    )
                return et

            def emit_attn(et, v_aug):
                attn_n = anp.tile([128, 4, 512], BF16, tag="attn_n")
                attnT = atp.tile([128, 4, 4, 128], BF16, tag="attnT")
                for c in range(4):
                    for g in range(2):
                        pa = pau.tile([128, 4, 66], F32, tag="attnU")
                        for hh in range(4):
                            h = g * 4 + hh
                            nc.tensor.matmul(
                                out=pa[:, hh, :],
                                lhsT=et[:, h, c * 128:(c + 1) * 128],
                                rhs=v_aug[:, h, :],
                                start=True, stop=True,
                            )
                        rr = rrp.tile([128, 4, 1], F32, tag="rr")
                        nc.vector.reciprocal(out=rr, in_=pa[:, :, 64:65])
                        out_ap = attn_n[:, c, g * 256:(g + 1) * 256].rearrange(
                            "p (h d) -> p h d", h=4
                        )
                        in0, in1 = broadcast_tensor_aps(pa[:, :, 0:64], rr)
                        nc.vector.tensor_mul(out_ap, in0, in1)
                    nc.sync.dma_start_transpose(
                        out=attnT[:, c, :, :], in_=attn_n[:, c, :],
                    )
                return attnT

            def emit_outproj(attnT, k, last=False):
                b, s0 = bs(k)
                osb = osp.tile([128, 4, 512], F32, tag="osb")
                for c in range(4):
                    po = pbig.tile([128, 512], F32, tag="big")
                    for j in range(4):
                        nc.tensor.matmul(
                            out=po,
                            lhsT=attnT[:, c, j, :],
                            rhs=wo_sb[:, j, :],
                            start=(j == 0), stop=(j == 3),
                        )
                    nc.vector.tensor_add(osb[:, c, :], po, bias_b)
                    if last:
                        # drain-time tile: store chunk-wise on SP so the
                        # final transfer overlaps the remaining adds
                        nc.sync.dma_start(
                            out=out_d[b, s0 + c * 128:s0 + (c + 1) * 128, :],
                            in_=osb[:, c, :],
                        )
                if not last:
                    nc.gpsimd.dma_start(
                        out=out_d[b, s0:s0 + 512, :].rearrange("(t p) d -> p t d", p=128),
                        in_=osb,
                    )

            # ---- software-pipelined main loop ----
            # st[k] = dict of live per-tile objects
            st = {}
            kv_of = {}  # tile index -> (kT_sb, v_aug)

            # prologue, in DMA service order: ctx(0) (tiny, kv-critical),
            # then Wk/Wv (kv matmuls), then x(0)/Wq (first qT), then the rest
            ctxT0 = emit_ctx(0)
            nc.gpsimd.dma_start(out=wk_sb, in_=wk_d[:].rearrange("(c p) e -> p c e", p=128))
            st[0] = {"x": emit_xload(0)}
            nc.gpsimd.dma_start(out=wq_sb, in_=wq_d[:].rearrange("(c p) e -> p c e", p=128))
            st[0]["xT"] = emit_xT(st[0]["x"])
            nc.gpsimd.dma_start(out=wv_sb, in_=wv_d[:].rearrange("(c p) e -> p c e", p=128))
            kv = emit_kv(ctxT0)
            st[1] = {"x": emit_xload(1)}
            nc.gpsimd.dma_start(out=wo_sb, in_=wo_d[:].rearrange("(c p) e -> p c e", p=128))
            nc.gpsimd.dma_start(out=bias_b, in_=bo_d[:].partition_broadcast(128))

            for k in range(NTILES + 2):
                # stage A: next-next x load
                if k + 2 < NTILES:
                    st[k + 2] = {"x": emit_xload(k + 2)}
                # stage B: next xT transpose
                if 0 < k + 1 < NTILES:
                    st[k + 1]["xT"] = emit_xT(st[k + 1]["x"])

                if k < NTILES:
                    b = k // NT
                    # prefetch next batch's ctx early and its k/v mid-batch so
                    # the PE work and ctx DMA land before the batch boundary
                    if k % NT == 2 and b + 1 < BPC:
                        ctxT_next = emit_ctx(b + 1)
                    if k % NT == NT - 3 and b + 1 < BPC:
                        kv_next = emit_kv(ctxT_next)
                    if k % NT == 0 and k > 0:
                        kv = kv_next
                    kv_of[k] = kv
                    # PE stage 1: qT(k)
                    st[k]["qT"] = emit_qT(st[k]["xT"])
                    # PE stage 2: scores+exp(k)
                    st[k]["et"] = emit_scores(st[k]["qT"], kv_of[k][0])
                # PE stage 3: attnU/norm/transpose(k-1)
                if 0 <= k - 1 < NTILES and "attnT" not in st[k - 1]:
                    st[k - 1]["attnT"] = emit_attn(st[k - 1]["et"], kv_of[k - 1][1])
                # epilogue shortcut: run the last tile's attention stage
                # lag-0 (its exps are long done by this point in the PE
                # stream), and ahead of outproj(k-2) so its norms win the
                # DVE queue — the final outproj then isn't stuck behind a
                # fresh DMA transpose at drain time
                if k == NTILES - 1:
                    st[k]["attnT"] = emit_attn(st[k]["et"], kv_of[k][1])
                # PE stage 4: outproj(k-2)
                if 0 <= k - 2:
                    emit_outproj(st[k - 2]["attnT"], k - 2, last=(k - 2 == NTILES - 1))
                    del st[k - 2]

    # TRN2 hardware allows at most 1 semaphore wait per instruction; split
    # multi-wait instructions into standalone EventSemaphore waits.
    _bass_rust.generate_event_semaphores(nc)
    return nc


_NC_CACHE = None


def kernel(x, context, Wq, Wk, Wv, Wout, bout):
    global _NC_CACHE
    if _NC_CACHE is None:
        _NC_CACHE = build_nc()
    nc = _NC_CACHE

    f = lambda a: np.ascontiguousarray(np.asarray(a), dtype=np.float32)
    x, context = f(x), f(context)
    Wq, Wk, Wv, Wout, bout = f(Wq), f(Wk), f(Wv), f(Wout), f(bout)

    in_maps = [
        {
            "x": x[c * BPC:(c + 1) * BPC],
            "context": context[c * BPC:(c + 1) * BPC],
            "Wq": Wq, "Wk": Wk, "Wv": Wv, "Wout": Wout, "bout": bout,
        }
        for c in range(N_CORES)
    ]
    res = run_bass_kernel_spmd(nc, in_maps, core_ids=list(range(N_CORES)))
    return np.concatenate([r["out"] for r in res.results], axis=0)
